# revision 18
# baseline (speedup 1.0000x reference)
"""Causal single-head attention (B=4, T=4096, E=1024, H=64) on 8 trn2 cores.

reference:
    q,k,v = x@Wq, x@Wk, x@Wv          # per batch
    s = q @ k.T  (causal masked)
    out = (softmax(s) / sqrt(64)) @ v

Sharding: core c = 2*b + s handles batch b; queries are striped by 128-row
blocks (core s owns global q-blocks j with j % 2 == s).  The host rolls each
core's copy of X down by 128*s rows, which makes every core's program
identical: own queries are the local-EVEN 128-blocks, and the causal
structure relative to local coordinates is core-independent.  The rolled-away
first block re-enters as local block 31 ("wrap" block); its mask is all-ones
for s=1 (those keys precede everything) and all-zeros for s=0 (handled by
the normal diagonal path instead) — shipped as per-core mask data.

Per core: stream X (32 row-tiles) -> PE-transpose X tiles -> fp32r matmuls
for K^T [64,4096], V^T -> V (PE transpose) [4096,64]+denominator column of
8.0, Q^T for own queries [64,2048]; then per 256-query macro-tile: S^T =
K_blk @ Q^T in PSUM, exp on ACT (with -30 bias for overflow margin; cancels
in the softmax ratio), causal masks via 0/1 multiplies, P^T @ V_aug
accumulated in PSUM (ones-column yields the softmax denominator * 8, folding
the /sqrt(64)), transpose back, divide, write own output rows.
"""
import sys

if "/opt/trn_rl_repo" not in sys.path:
    sys.path.insert(0, "/opt/trn_rl_repo")

import numpy as np

import concourse.bass as bass
import concourse.tile as tile
from concourse import mybir
from concourse.vector_clock import ScopedClock, VectorClock

B = 4
T = 4096
E = 1024
H = 64
NCORES = 8
NG = 8          # groups of 512 rows
GR = 512        # rows per group
NEC = 8         # 128-wide chunks of E
NQB = 32        # 128-row blocks of T
EXP_BIAS = -30.0

F32 = mybir.dt.float32
F32R = mybir.dt.float32r

_cache = {}


# ---------------------------------------------------------------------------
# Walrus in this container rejects >1 sync wait per instruction and any DMA
# sem wait on SP/TPB_CTRL instructions.  Carry the kernel-tail waits on dummy
# scalar-engine copies (one wait each) instead of the drain itself, and issue
# every DMA from gpsimd so body DMA waits never land on SP.
# ---------------------------------------------------------------------------
class SplitDrainTileContext(tile.TileContext):
    def _drain_and_barrier(self, tick_clock, wait_clock):
        dummy = self.nc._tail_drain_dummy_ap
        gc = tick_clock.global_clock
        n = len(gc)
        for p in [i for i in range(n) if gc[i] > 0]:
            vec = [0] * n
            vec[p] = gc[p]
            carrier = self.nc.scalar.copy(dummy[:, :], dummy[:, :])
            wait_clock.add_sem_waits(
                carrier.ins, ScopedClock({None: VectorClock(vec)})
            )
        self.nc.sync.drain()

        self.nc.all_engine_barrier()
        assert self.sems is not None
        popped = self.nc._tile_sem_poison_stack.pop()
        assert popped is self._sem_poison
        self.nc.clear_and_free_semaphores(list(self.sems.allocated().values()))
        self.nc.all_engine_barrier()


def split_multi_waits(nc):
    """Walrus here encodes at most ONE sync wait per instruction.  Move the
    extra waits of any multi-wait instruction onto same-engine NOPs placed
    immediately before it (program order on the engine preserves semantics)."""
    for f in nc.m.functions:
        for bb in f.blocks:
            insts = bb.instructions
            i = 0
            while i < len(insts):
                inst = insts[i]
                si = inst.sync_info
                if si is not None and si.on_wait and len(si.on_wait) > 1:
                    waits = list(si.on_wait)
                    for w in waits[:-1]:
                        nop = mybir.InstNoOp(
                            name=f"I-{nc.next_id()}", ins=[], outs=[]
                        )
                        nop.engine = inst.engine
                        nop.sync_info = mybir.SyncInfo(
                            on_wait=[w], on_update=[]
                        )
                        nc.register_instruction(nop)
                        insts.insert(i, nop)
                        i += 1
                    inst.sync_info = mybir.SyncInfo(
                        on_wait=[waits[-1]], on_update=list(si.on_update)
                    )
                i += 1


def build_kernel():
    nc = bass.Bass("TRN2", target_bir_lowering=False, debug=False)
    nc._tail_drain_dummy_ap = nc.alloc_sbuf_tensor(
        "tail_drain_dummy", [1, 1], F32
    ).ap()

    x = nc.dram_tensor("x", [T, E], F32, kind="ExternalInput").ap()
    wk = nc.dram_tensor("wk", [E, H], F32, kind="ExternalInput").ap()
    wq = nc.dram_tensor("wq", [E, H], F32, kind="ExternalInput").ap()
    wv = nc.dram_tensor("wv", [E, H], F32, kind="ExternalInput").ap()
    masks = nc.dram_tensor("masks", [128, 1024], F32, kind="ExternalInput").ap()
    ident = nc.dram_tensor("ident", [128, 128], F32, kind="ExternalInput").ap()
    consts = nc.dram_tensor("consts", [128, 33], F32, kind="ExternalInput").ap()
    out = nc.dram_tensor("out", [T // 2, H], F32, kind="ExternalOutput").ap()

    with SplitDrainTileContext(nc) as tc:
        _build_body(nc, tc, x, wk, wq, wv, masks, ident, consts, out)
    split_multi_waits(nc)
    return nc


def _build_body(nc, tc, x, wk, wq, wv, masks, ident, consts, out):
    from contextlib import ExitStack

    ctx = ExitStack()
    with ctx:
        const_pool = ctx.enter_context(tc.tile_pool(name="const", bufs=1))
        persist = ctx.enter_context(tc.tile_pool(name="persist", bufs=1))
        xg_pool = ctx.enter_context(tc.tile_pool(name="xg", bufs=2))
        xt_pool = ctx.enter_context(tc.tile_pool(name="xt", bufs=16))
        exps_pool = ctx.enter_context(tc.tile_pool(name="exps", bufs=3))
        small_pool = ctx.enter_context(tc.tile_pool(name="small", bufs=4))

        # ---- constants -> SBUF
        wk_sb = const_pool.tile([128, NEC, H], F32R)
        nc.gpsimd.dma_start(wk_sb[:], wk.rearrange("(c p) w -> p c w", p=128))
        wq_sb = const_pool.tile([128, NEC, H], F32R)
        nc.gpsimd.dma_start(wq_sb[:], wq.rearrange("(c p) w -> p c w", p=128))
        wv_sb = const_pool.tile([128, NEC, H], F32R)
        nc.gpsimd.dma_start(wv_sb[:], wv.rearrange("(c p) w -> p c w", p=128))
        masks_sb = const_pool.tile([128, 1024], F32)
        nc.gpsimd.dma_start(masks_sb[:], masks[:])
        id_sb = const_pool.tile([128, 128], F32)
        nc.gpsimd.dma_start(id_sb[:], ident[:])

        # ---- persistent intermediates
        kt_all = persist.tile([64, T], F32R)          # K^T
        qt_all = persist.tile([64, T // 2], F32R)     # Q^T, own queries
        vaug = persist.tile([128, NQB * (H + 1)], F32R)  # V blocks + denom col
        outstage = persist.tile([128, 16 * H], F32)

        # denominator column = 8.0 (folds the /sqrt(64)); DMA-cast from host
        nc.gpsimd.dma_start(
            vaug[:].rearrange("p (i c) -> p i c", c=H + 1)[:, :, H : H + 1],
            consts[:, 0:32].rearrange("p (i u) -> p i u", u=1),
        )

        # per-partition bias column for exp(s + EXP_BIAS)
        expbias = const_pool.tile([128, 1], F32)
        nc.gpsimd.dma_start(expbias[:], consts[:, 32:33])

        # ================= phase P: projections =================
        pp = ExitStack()
        with pp:
            xt_ps_pool = pp.enter_context(
                tc.tile_pool(name="xt_ps", bufs=2, space="PSUM")
            )
            kt_ps_pool = pp.enter_context(
                tc.tile_pool(name="kt_ps", bufs=2, space="PSUM")
            )
            vt_ps_pool = pp.enter_context(
                tc.tile_pool(name="vt_ps", bufs=2, space="PSUM")
            )
            qt_ps_pool = pp.enter_context(
                tc.tile_pool(name="qt_ps", bufs=1, space="PSUM")
            )
            vtr_ps_pool = pp.enter_context(
                tc.tile_pool(name="vtr_ps", bufs=1, space="PSUM")
            )

            for g in range(NG):
                xg = xg_pool.tile([128, 4 * E], F32)
                nc.gpsimd.dma_start(
                    xg[:].rearrange("p (a m) -> p a m", a=4),
                    x[g * GR : (g + 1) * GR, :].rearrange(
                        "(a p) m -> p a m", p=128
                    ),
                )

                # transpose X group: xt[ec] = X[group rows, ec-chunk].T
                xts = []
                for ec in range(NEC):
                    xt_ps = xt_ps_pool.tile([128, GR], F32)
                    for a in range(4):
                        nc.tensor.transpose(
                            xt_ps[:, a * 128 : (a + 1) * 128],
                            xg[:, a * E + ec * 128 : a * E + (ec + 1) * 128],
                            id_sb[:],
                        )
                    xt_sb = xt_pool.tile([128, GR], F32R)
                    eng = nc.vector.tensor_copy if ec % 2 == 0 else nc.scalar.copy
                    eng(xt_sb[:], xt_ps[:])
                    xts.append(xt_sb)

                # K^T chain
                kt_ps = kt_ps_pool.tile([64, GR], F32)
                for ec in range(NEC):
                    nc.tensor.matmul(
                        kt_ps[:],
                        wk_sb[:, ec, :],
                        xts[ec][:],
                        start=(ec == 0),
                        stop=(ec == NEC - 1),
                    )
                nc.vector.tensor_copy(
                    kt_all[:, g * GR : (g + 1) * GR], kt_ps[:]
                )

                # V^T chain
                vt_ps = vt_ps_pool.tile([64, GR], F32)
                for ec in range(NEC):
                    nc.tensor.matmul(
                        vt_ps[:],
                        wv_sb[:, ec, :],
                        xts[ec][:],
                        start=(ec == 0),
                        stop=(ec == NEC - 1),
                    )
                vt_sb = small_pool.tile([64, GR], F32, tag="vt_sb")
                nc.scalar.copy(vt_sb[:], vt_ps[:])

                # V natural blocks (4 per group) via PE transpose
                for a in range(4):
                    vtr_ps = vtr_ps_pool.tile([128, H], F32)
                    nc.tensor.transpose(
                        vtr_ps[:],
                        vt_sb[:, a * 128 : (a + 1) * 128],
                        id_sb[0:64, 0:64],
                    )
                    i = 4 * g + a
                    nc.vector.tensor_copy(
                        vaug[:, i * (H + 1) : i * (H + 1) + H], vtr_ps[:]
                    )

                # Q^T chain: own queries are local blocks 4g and 4g+2
                # (columns 0:128 and 256:384 of the group); one N=256 matmul
                # per e-chunk via a strided 3D rhs AP
                qt_ps = qt_ps_pool.tile([64, 256], F32)
                for ec in range(NEC):
                    rhs = xts[ec][:].rearrange(
                        "p (a r m) -> p r a m", a=2, r=2
                    )[:, 0, :, :]
                    nc.tensor.matmul(
                        qt_ps[:].rearrange("p (a m) -> p a m", a=2),
                        wq_sb[:, ec, :],
                        rhs,
                        start=(ec == 0),
                        stop=(ec == NEC - 1),
                    )
                nc.vector.tensor_copy(
                    qt_all[:, g * 256 : (g + 1) * 256], qt_ps[:]
                )

        # ================= phase A: attention =================
        pa = ExitStack()
        with pa:
            s_ps_pool = pa.enter_context(
                tc.tile_pool(name="s_ps", bufs=2, space="PSUM")
            )
            av_ps_pool = pa.enter_context(
                tc.tile_pool(name="av_ps", bufs=2, space="PSUM")
            )
            ot_ps_pool = pa.enter_context(
                tc.tile_pool(name="ot_ps", bufs=2, space="PSUM")
            )

            for jj in range(NG):
                # key blocks for this 256-query macro-tile: 4jj full blocks,
                # then diag d=0,1,2 (local blocks 4jj..4jj+2), then wrap (31)
                kblocks = list(range(4 * jj + 3)) + [31]
                nk = len(kblocks)  # 4jj+4
                nchunks = (nk + 3) // 4  # = jj+1
                qs = qt_all[:, jj * 256 : (jj + 1) * 256]

                av_ps = av_ps_pool.tile([H + 1, 256], F32)
                pend = None  # (exps tile, chunk kblock list, first flag)
                first_av = True
                for c in range(nchunks):
                    blocks = kblocks[c * 4 : (c + 1) * 4]
                    s_ps = s_ps_pool.tile([128, 1024], F32)
                    for ci, i in enumerate(blocks):
                        nc.tensor.matmul(
                            s_ps[:, ci * 256 : (ci + 1) * 256],
                            kt_all[:, i * 128 : (i + 1) * 128],
                            qs,
                            start=True,
                            stop=True,
                        )
                    exps = exps_pool.tile([128, 1024], F32R)
                    n = len(blocks) * 256
                    nc.scalar.activation(
                        exps[:, 0:n],
                        s_ps[:, 0:n],
                        mybir.ActivationFunctionType.Exp,
                        bias=expbias[:, 0:1],
                    )
                    if c == nchunks - 1:
                        # masked chunk: [d0 | d1 | d2 | wrap] 0/1 masks
                        nc.vector.tensor_mul(
                            exps[:, 0:1024], exps[:, 0:1024], masks_sb[:, 0:1024]
                        )
                    # issue previous chunk's AV now (keeps PE busy during exp)
                    if pend is not None:
                        pexps, pblocks = pend
                        for ci, i in enumerate(pblocks):
                            nc.tensor.matmul(
                                av_ps[:],
                                vaug[:, i * (H + 1) : (i + 1) * (H + 1)],
                                pexps[:, ci * 256 : (ci + 1) * 256],
                                start=first_av,
                                stop=False,
                            )
                            first_av = False
                    pend = (exps, blocks)
                pexps, pblocks = pend
                for ci, i in enumerate(pblocks):
                    nc.tensor.matmul(
                        av_ps[:],
                        vaug[:, i * (H + 1) : (i + 1) * (H + 1)],
                        pexps[:, ci * 256 : (ci + 1) * 256],
                        start=first_av,
                        stop=(ci == len(pblocks) - 1),
                    )
                    first_av = False

                # out^T [65, 256] -> out [256, 65] -> divide -> stage
                avs = small_pool.tile([H + 1, 256], F32, tag="avs")
                nc.scalar.copy(avs[:], av_ps[:])
                for t in range(2):
                    ot_ps = ot_ps_pool.tile([128, H + 1], F32)
                    nc.tensor.transpose(
                        ot_ps[:],
                        avs[:, t * 128 : (t + 1) * 128],
                        id_sb[0 : H + 1, 0 : H + 1],
                    )
                    rcp = small_pool.tile([128, 1], F32, tag="rcp")
                    nc.vector.reciprocal(rcp[:], ot_ps[:, H : H + 1])
                    q128 = 2 * jj + t
                    nc.vector.tensor_scalar_mul(
                        outstage[:, q128 * H : (q128 + 1) * H],
                        ot_ps[:, 0:H],
                        rcp[:],
                    )

        nc.gpsimd.dma_start(
            out.rearrange("(b p) h -> p b h", p=128),
            outstage[:].rearrange("p (b h) -> p b h", h=H),
        )


def _host_inputs(input, Wq, Wk, Wv):
    """Build the 8 per-core input maps from the full problem inputs."""
    triu = np.triu(np.ones((128, 128), dtype=np.float32))
    ones = np.ones((128, 128), dtype=np.float32)
    zeros = np.zeros((128, 128), dtype=np.float32)
    ident = np.eye(128, dtype=np.float32)
    consts = np.empty((128, 33), dtype=np.float32)
    consts[:, 0:32] = 8.0
    consts[:, 32] = EXP_BIAS

    in_maps = []
    for c in range(NCORES):
        b, s = divmod(c, 2)
        xb = np.asarray(input[b])
        x_rot = np.roll(xb, -128 * s, axis=0) if s else xb
        wrap = ones if s == 1 else zeros
        m = np.concatenate(
            [triu, ones, zeros, ones, zeros, triu, wrap, wrap], axis=1
        )  # [128, 1024] = d0 | d1 | d2 | wrap
        in_maps.append(
            {
                "x": np.ascontiguousarray(x_rot, dtype=np.float32),
                "wk": np.asarray(Wk, dtype=np.float32),
                "wq": np.asarray(Wq, dtype=np.float32),
                "wv": np.asarray(Wv, dtype=np.float32),
                "masks": np.ascontiguousarray(m),
                "ident": ident,
                "consts": consts,
            }
        )
    return in_maps


def _assemble(results):
    """Scatter per-core striped outputs back to [B, T, H]."""
    out = np.empty((B, T, H), dtype=np.float32)
    for c in range(NCORES):
        b, s = divmod(c, 2)
        o = results[c]["out"].reshape(16, 128, H)  # own blocks, in order
        view = out[b].reshape(32, 128, H)
        view[s::2] = o
    return out


def kernel(input, Wq, Wk, Wv):
    from concourse.bass_utils import run_bass_kernel_spmd

    if "nc" not in _cache:
        _cache["nc"] = build_kernel()
    nc = _cache["nc"]
    in_maps = _host_inputs(input, Wq, Wk, Wv)
    res = run_bass_kernel_spmd(nc, in_maps, core_ids=list(range(NCORES)))
    return _assemble(res.results)
